# revision 18
# baseline (speedup 1.0000x reference)
# KL divergence loss kernel for Trainium2 (Bass/Tile), 8-core data-parallel.
#
# Problem: KL(p||q) for diagonal Gaussians over [B=16, L=64, N=512, D=64] f32
# tensors, reduced to a scalar: mean over (B,L) of sum over (N,D) of
#   log(qs/ps) + 0.5*(ps^2 + (pm-qm)^2)/qs^2 - 0.5
#
# Strategy (pure data-parallel, hardcoded):
#   - Shard along B: core c gets B-rows [2c, 2c+2) -> [2,64,512,64], viewed as
#     [128 partitions, 32768 free] (partition = (b,l) pair, free = (n,d)).
#   - Per core, stream tiles of [128, F] per tensor through SBUF. The DMA
#     engines are the bottleneck (16 engines x ~26.3 GB/s = ~421 GB/s/core);
#     everything else hides behind the stream. Tile widths shrink at the end
#     (2048 ... 1024, 512, 512) so the final tile's serial compute chain after
#     the last DMA byte lands is short.
#   - Math restructured to avoid division (ACT Reciprocal is blocked):
#       w  = exp(-ln(qs))            = 1/qs      (DVE reciprocal_approx_fast)
#       r1 = ps * w, d = pm - qm, r2 = d * w     (DVE / GpSimd)
#       S1 = sum ln(r1) = -sum log-ratio          (ACT Ln with free accum_out)
#       S2 = sum r1^2, S3 = sum r2^2              (ACT Square with free accum_out)
#     r1 and r2 land in pools separate from the io pool: a DVE tensor-tensor
#     mul whose output shares the io pool's slot alignment runs ~2.4x slower.
#   - Accumulators are interleaved per iteration (cols 3i..3i+2). All but the
#     last two iterations' columns DMA out early (overlapped with the tail);
#     only 6 columns ship after the final accum read.
#   - Iteration 0's four loads issue on both HWDGE rings (Sync + Scalar) to
#     shorten the DMA ramp; steady state issues on Sync only.
#   - Host combines partial sums in float64:
#       mean = (-S1 + 0.5*(S2+S3))/(B*L) - N*D/2.

import numpy as np

B, L, N, D = 16, 64, 512, 64
NCORES = 8
P = 128                      # SBUF partitions = per-core B*L = (B/NCORES)*L
TOT = N * D                  # free elements per partition = 32768
FS = [2048] * 15 + [1024, 1024]       # tile widths; sum == TOT
assert sum(FS) == TOT
NIT = len(FS)

_CACHE = {}


def build_nc():
    from contextlib import ExitStack
    import concourse.tile as tile
    from concourse import bacc, mybir

    dt = mybir.dt.float32
    AF = mybir.ActivationFunctionType

    nc = bacc.Bacc(
        "TRN2", target_bir_lowering=False, debug=False, num_devices=NCORES
    )
    pm = nc.dram_tensor("prior_mu", [P, TOT], dt, kind="ExternalInput").ap()
    ps = nc.dram_tensor("prior_sigma", [P, TOT], dt, kind="ExternalInput").ap()
    qm = nc.dram_tensor("post_mu", [P, TOT], dt, kind="ExternalInput").ap()
    qs = nc.dram_tensor("post_sigma", [P, TOT], dt, kind="ExternalInput").ap()
    out = nc.dram_tensor("acc_out", [P, 3 * NIT], dt, kind="ExternalOutput").ap()

    NEARLY = 3 * (NIT - 2)   # acc columns shipped by the early DMA

    with tile.TileContext(nc) as tc, ExitStack() as ctx:
        io = ctx.enter_context(tc.tile_pool(name="io", bufs=4))
        r1p = ctx.enter_context(tc.tile_pool(name="r1p", bufs=3))
        r2p = ctx.enter_context(tc.tile_pool(name="r2p", bufs=3))
        accp = ctx.enter_context(tc.tile_pool(name="accp", bufs=1))
        acc = accp.tile([P, 3 * NIT], dt)
        # ACT needs a full-size out even when only accum_out matters; park it
        # in one scratch tile (WAW on ACT only — sequential there anyway).
        scr = accp.tile([P, 2048], dt)

        # All square-sums stay on ACT. Moving the tail iterations' squares
        # to DVE (scalar_tensor_tensor or tensor_tensor_reduce with
        # accum_out) would shorten the ACT-serialized tail on paper, but any
        # change to DVE's op mix re-rolls the tile scheduler's global order
        # and re-triggered the slow ~12.5us/iter equilibrium (measured 213us
        # and 216us vs 184us); tensor_tensor_reduce also hit runtime
        # INTERNAL errors on HW.
        def sq_accum(src, j, col, Fj):
            nc.scalar.activation(
                scr[:, :Fj], src[:], AF.Square,
                accum_out=acc[:, col:col + 1],
            )

        # Software-pipelined by one iteration: iteration i runs its own
        # r1 chain (recip/r1/Ln/Sq1) but iteration i-1's r2 chain (r2 mul +
        # Sq2). Each iteration loads pm/qm FIRST so the GpSimd subtract's
        # inputs land two DMA-slots before qs/ps: the subtract finishes
        # before the in-order DVE stream (which the tile scheduler orders as
        # [r1mul(i), r2mul(i-1 or i), recip(i+1), ...]) ever reaches the r2
        # mul. If the subtract instead waits on the LAST DMA of its
        # iteration, DVE stalls mid-stream, io-slot releases slip, the DMA
        # issue window tightens, and the whole loop settles into a slow
        # ~12.5us/iter equilibrium (~35% below the DMA roofline).
        pipe = []  # [(qs_t, pm_t, j, Fj)] pending r2 stages, oldest first

        def r2_stage(p):
            qs_p, pm_p, j, Fj = p
            # r2 = d*w, then S3 += sum r2^2
            r2_t = r2p.tile([P, Fj], dt)
            nc.vector.tensor_mul(r2_t[:], pm_p[:], qs_p[:])
            sq_accum(r2_t, j, 3 * j + 2, Fj)

        off = 0
        for i, F in enumerate(FS):
            sl = np.s_[:, off:off + F]
            off += F
            pm_t = io.tile([P, F], dt)
            qm_t = io.tile([P, F], dt)
            qs_t = io.tile([P, F], dt)
            ps_t = io.tile([P, F], dt)
            if i == 0:
                # Both HWDGE rings for the ramp: Sync + Scalar.
                nc.sync.dma_start(pm_t[:], pm[sl])
                nc.scalar.dma_start(qs_t[:], qs[sl])
                nc.sync.dma_start(qm_t[:], qm[sl])
                nc.scalar.dma_start(ps_t[:], ps[sl])
            else:
                nc.sync.dma_start(pm_t[:], pm[sl])
                nc.sync.dma_start(qm_t[:], qm[sl])
                nc.sync.dma_start(qs_t[:], qs[sl])
                nc.sync.dma_start(ps_t[:], ps[sl])

            # d = pm - qm on the otherwise-idle GpSimd engine
            nc.gpsimd.tensor_sub(pm_t[:], pm_t[:], qm_t[:])
            # w = 1/qs, in place in qs_t (single custom-DVE op, ~51 ULP)
            nc.vector.reciprocal_approx_fast(out=qs_t[:], in_=qs_t[:])
            # r1 = ps*w -> dedicated tile
            r1_t = r1p.tile([P, F], dt)
            nc.vector.tensor_mul(r1_t[:], ps_t[:], qs_t[:])
            if len(pipe) >= 1:
                r2_stage(pipe.pop(0))
            if i == NIT - 1:
                # Ship all but the last two iterations' accumulators while
                # the tail compute still runs; deps resolve via Scalar's
                # in-order accum reads (Sq2(NIT-2) just issued above).
                nc.scalar.dma_start(out[:, :NEARLY], acc[:, :NEARLY])
            # S1 += sum ln(r1) ; S2 += sum r1^2
            # (Ln and Square share one ACT table set -> single table load)
            nc.scalar.activation(
                scr[:, :F], r1_t[:], AF.Ln, accum_out=acc[:, 3 * i:3 * i + 1]
            )
            sq_accum(r1_t, i, 3 * i + 1, F)
            pipe.append((qs_t, pm_t, i, F))

        for p in pipe:
            r2_stage(p)
        nc.sync.dma_start(out[:, NEARLY:], acc[:, NEARLY:])

    nc.compile()
    return nc


def _shard(a, c):
    a = np.asarray(a, dtype=np.float32)
    return np.ascontiguousarray(a[2 * c:2 * c + 2]).reshape(P, TOT)


def make_in_maps(prior_mu, prior_sigma, post_mu, post_sigma):
    return [
        {
            "prior_mu": _shard(prior_mu, c),
            "prior_sigma": _shard(prior_sigma, c),
            "post_mu": _shard(post_mu, c),
            "post_sigma": _shard(post_sigma, c),
        }
        for c in range(NCORES)
    ]


def combine(results):
    S1 = S2 = S3 = 0.0
    for r in results:
        a = r["acc_out"].astype(np.float64).reshape(P, NIT, 3)
        S1 += a[:, :, 0].sum()
        S2 += a[:, :, 1].sum()
        S3 += a[:, :, 2].sum()
    mean = (-S1 + 0.5 * (S2 + S3)) / (B * L) - 0.5 * N * D
    return np.float32(mean)


def kernel(prior_mu, prior_sigma, post_mu, post_sigma):
    from concourse.bass_utils import run_bass_kernel_spmd

    if "nc" not in _CACHE:
        _CACHE["nc"] = build_nc()
    nc = _CACHE["nc"]
    in_maps = make_in_maps(prior_mu, prior_sigma, post_mu, post_sigma)
    res = run_bass_kernel_spmd(nc, in_maps, list(range(NCORES)))
    return combine(res.results)


# revision 19
# speedup vs baseline: 1.0917x; 1.0917x over previous
# KL divergence loss kernel for Trainium2 (Bass/Tile), 8-core data-parallel.
#
# Problem: KL(p||q) for diagonal Gaussians over [B=16, L=64, N=512, D=64] f32
# tensors, reduced to a scalar: mean over (B,L) of sum over (N,D) of
#   log(qs/ps) + 0.5*(ps^2 + (pm-qm)^2)/qs^2 - 0.5
#
# Strategy (pure data-parallel, hardcoded):
#   - Shard along B: core c gets B-rows [2c, 2c+2) -> [2,64,512,64], viewed as
#     [128 partitions, 32768 free] (partition = (b,l) pair, free = (n,d)).
#   - Per core, stream tiles of [128, F] per tensor through SBUF. The DMA
#     engines are the bottleneck (16 engines x ~26.3 GB/s = ~421 GB/s/core);
#     everything else hides behind the stream. Tile widths shrink at the end
#     (2048 ... 1024, 512, 512) so the final tile's serial compute chain after
#     the last DMA byte lands is short.
#   - Math restructured to avoid division (ACT Reciprocal is blocked):
#       w  = exp(-ln(qs))            = 1/qs      (DVE reciprocal_approx_fast)
#       r1 = ps * w, d = pm - qm, r2 = d * w     (DVE / GpSimd)
#       S1 = sum ln(r1) = -sum log-ratio          (ACT Ln with free accum_out)
#       S2 = sum r1^2, S3 = sum r2^2              (ACT Square with free accum_out)
#     r1 and r2 land in pools separate from the io pool: a DVE tensor-tensor
#     mul whose output shares the io pool's slot alignment runs ~2.4x slower.
#   - Accumulators are interleaved per iteration (cols 3i..3i+2). All but the
#     last two iterations' columns DMA out early (overlapped with the tail);
#     only 6 columns ship after the final accum read.
#   - Iteration 0's four loads issue on both HWDGE rings (Sync + Scalar) to
#     shorten the DMA ramp; steady state issues on Sync only.
#   - Host combines partial sums in float64:
#       mean = (-S1 + 0.5*(S2+S3))/(B*L) - N*D/2.

import numpy as np

B, L, N, D = 16, 64, 512, 64
NCORES = 8
P = 128                      # SBUF partitions = per-core B*L = (B/NCORES)*L
TOT = N * D                  # free elements per partition = 32768
FS = [2048] * 15 + [1024, 512, 512]   # tile widths; sum == TOT
assert sum(FS) == TOT
NIT = len(FS)

_CACHE = {}


def build_nc():
    from contextlib import ExitStack
    import concourse.tile as tile
    from concourse import bacc, mybir

    dt = mybir.dt.float32
    AF = mybir.ActivationFunctionType

    nc = bacc.Bacc(
        "TRN2", target_bir_lowering=False, debug=False, num_devices=NCORES
    )
    pm = nc.dram_tensor("prior_mu", [P, TOT], dt, kind="ExternalInput").ap()
    ps = nc.dram_tensor("prior_sigma", [P, TOT], dt, kind="ExternalInput").ap()
    qm = nc.dram_tensor("post_mu", [P, TOT], dt, kind="ExternalInput").ap()
    qs = nc.dram_tensor("post_sigma", [P, TOT], dt, kind="ExternalInput").ap()
    out = nc.dram_tensor("acc_out", [P, 3 * NIT], dt, kind="ExternalOutput").ap()

    NEARLY = 3 * (NIT - 2)   # acc columns shipped by the early DMA

    with tile.TileContext(nc) as tc, ExitStack() as ctx:
        io = ctx.enter_context(tc.tile_pool(name="io", bufs=4))
        r1p = ctx.enter_context(tc.tile_pool(name="r1p", bufs=3))
        r2p = ctx.enter_context(tc.tile_pool(name="r2p", bufs=3))
        accp = ctx.enter_context(tc.tile_pool(name="accp", bufs=1))
        acc = accp.tile([P, 3 * NIT], dt)
        # ACT needs a full-size out even when only accum_out matters; park it
        # in one scratch tile (WAW on ACT only — sequential there anyway).
        scr = accp.tile([P, 2048], dt)

        # All square-sums stay on ACT. Moving the tail iterations' squares
        # to DVE (scalar_tensor_tensor or tensor_tensor_reduce with
        # accum_out) would shorten the ACT-serialized tail on paper, but any
        # change to DVE's op mix re-rolls the tile scheduler's global order
        # and re-triggered the slow ~12.5us/iter equilibrium (measured 213us
        # and 216us vs 184us); tensor_tensor_reduce also hit runtime
        # INTERNAL errors on HW.
        def sq_accum(src, j, col, Fj):
            nc.scalar.activation(
                scr[:, :Fj], src[:], AF.Square,
                accum_out=acc[:, col:col + 1],
            )

        # Software-pipelined by one iteration: iteration i runs its own
        # r1 chain (recip/r1/Ln/Sq1) but iteration i-1's r2 chain (r2 mul +
        # Sq2). Each iteration loads pm/qm FIRST so the GpSimd subtract's
        # inputs land two DMA-slots before qs/ps: the subtract finishes
        # before the in-order DVE stream (which the tile scheduler orders as
        # [r1mul(i), r2mul(i-1 or i), recip(i+1), ...]) ever reaches the r2
        # mul. If the subtract instead waits on the LAST DMA of its
        # iteration, DVE stalls mid-stream, io-slot releases slip, the DMA
        # issue window tightens, and the whole loop settles into a slow
        # ~12.5us/iter equilibrium (~35% below the DMA roofline).
        pipe = []  # [(qs_t, pm_t, j, Fj)] pending r2 stages, oldest first

        def r2_stage(p):
            qs_p, pm_p, j, Fj = p
            # r2 = d*w, then S3 += sum r2^2
            r2_t = r2p.tile([P, Fj], dt)
            nc.vector.tensor_mul(r2_t[:], pm_p[:], qs_p[:])
            sq_accum(r2_t, j, 3 * j + 2, Fj)

        off = 0
        for i, F in enumerate(FS):
            sl = np.s_[:, off:off + F]
            off += F
            pm_t = io.tile([P, F], dt)
            qm_t = io.tile([P, F], dt)
            qs_t = io.tile([P, F], dt)
            ps_t = io.tile([P, F], dt)
            if i == 0:
                # Both HWDGE rings for the ramp: Sync + Scalar.
                nc.sync.dma_start(pm_t[:], pm[sl])
                nc.scalar.dma_start(qs_t[:], qs[sl])
                nc.sync.dma_start(qm_t[:], qm[sl])
                nc.scalar.dma_start(ps_t[:], ps[sl])
            else:
                nc.sync.dma_start(pm_t[:], pm[sl])
                nc.sync.dma_start(qm_t[:], qm[sl])
                nc.sync.dma_start(qs_t[:], qs[sl])
                nc.sync.dma_start(ps_t[:], ps[sl])

            # d = pm - qm on the otherwise-idle GpSimd engine
            nc.gpsimd.tensor_sub(pm_t[:], pm_t[:], qm_t[:])
            # w = 1/qs, in place in qs_t (single custom-DVE op, ~51 ULP)
            nc.vector.reciprocal_approx_fast(out=qs_t[:], in_=qs_t[:])
            # r1 = ps*w -> dedicated tile
            r1_t = r1p.tile([P, F], dt)
            nc.vector.tensor_mul(r1_t[:], ps_t[:], qs_t[:])
            if len(pipe) >= 1:
                r2_stage(pipe.pop(0))
            if i == NIT - 1:
                # Ship all but the last two iterations' accumulators while
                # the tail compute still runs; deps resolve via Scalar's
                # in-order accum reads (Sq2(NIT-2) just issued above).
                nc.scalar.dma_start(out[:, :NEARLY], acc[:, :NEARLY])
            # S1 += sum ln(r1) ; S2 += sum r1^2
            # (Ln and Square share one ACT table set -> single table load)
            nc.scalar.activation(
                scr[:, :F], r1_t[:], AF.Ln, accum_out=acc[:, 3 * i:3 * i + 1]
            )
            sq_accum(r1_t, i, 3 * i + 1, F)
            pipe.append((qs_t, pm_t, i, F))

        for p in pipe:
            r2_stage(p)
        nc.sync.dma_start(out[:, NEARLY:], acc[:, NEARLY:])

    nc.compile()
    return nc


def _shard(a, c):
    a = np.asarray(a, dtype=np.float32)
    return np.ascontiguousarray(a[2 * c:2 * c + 2]).reshape(P, TOT)


def make_in_maps(prior_mu, prior_sigma, post_mu, post_sigma):
    return [
        {
            "prior_mu": _shard(prior_mu, c),
            "prior_sigma": _shard(prior_sigma, c),
            "post_mu": _shard(post_mu, c),
            "post_sigma": _shard(post_sigma, c),
        }
        for c in range(NCORES)
    ]


def combine(results):
    S1 = S2 = S3 = 0.0
    for r in results:
        a = r["acc_out"].astype(np.float64).reshape(P, NIT, 3)
        S1 += a[:, :, 0].sum()
        S2 += a[:, :, 1].sum()
        S3 += a[:, :, 2].sum()
    mean = (-S1 + 0.5 * (S2 + S3)) / (B * L) - 0.5 * N * D
    return np.float32(mean)


def kernel(prior_mu, prior_sigma, post_mu, post_sigma):
    from concourse.bass_utils import run_bass_kernel_spmd

    if "nc" not in _CACHE:
        _CACHE["nc"] = build_nc()
    nc = _CACHE["nc"]
    in_maps = make_in_maps(prior_mu, prior_sigma, post_mu, post_sigma)
    res = run_bass_kernel_spmd(nc, in_maps, list(range(NCORES)))
    return combine(res.results)
